# revision 30
# baseline (speedup 1.0000x reference)
"""Trainium2 Bass kernel for AssociativeMemoryModule (causal linear attention).

Sharding: head-parallel — core c owns head c for both batches. Each core:
  1. projects full x (pre-transposed, bf16 on host) to [q.T;k.T] and v.T,
  2. applies phi = elu+1 via exp(min(.,0)) + relu(.) in f32,
  3. PE-transposes k_phi / v tiles to normal layout (bf16),
  4. runs chunked causal linear attention (C=128), normal-orientation
     epilogue (per-partition denominator), PE-transpose to out.T,
  5. per-batch AllToAll redistributes head-sharded -> t-sharded (bf16);
     batch-0's collective overlaps batch-1 compute, o-proj of batch 0
     overlaps batch-1's collective,
  6. each core computes the o-projection for its 128-col slice per batch.
Host reassembles the 8 (512, 256) o.T slices.
"""
import sys

import numpy as np

sys.path.insert(0, "/opt/trn_rl_repo")

H, HD, D = 8, 64, 512
B, T = 2, 1024
BT = B * T            # 2048
C = 128               # attention chunk
NCH = BT // C         # 16 chunks total
CPB = T // C          # 8 chunks per batch
TS = BT // 8          # 256: per-core output t-cols (128 per batch)
NF = D // 128         # 4 feature tiles
NT = 4                # t-tiles of 512 for projections
N_WARM = 90           # PE-warming dummy matmuls bridging the collective

_CACHE = {}


def _build():
    if "nc" in _CACHE:
        return _CACHE["nc"]
    import concourse.mybir as mybir
    import concourse.tile as tile
    from concourse import bacc
    from concourse.bass import ts

    import ml_dtypes

    f32 = mybir.dt.float32
    bf16 = mybir.dt.bfloat16
    AF = mybir.ActivationFunctionType
    ALU = mybir.AluOpType

    nc = bacc.Bacc("TRN2", target_bir_lowering=False, debug=False, num_devices=8)

    xT = nc.declare_dram_parameter("xT", [D, BT], bf16, isOutput=False)
    wa = nc.declare_dram_parameter("wa", [D, 128], bf16, isOutput=False)
    wv = nc.declare_dram_parameter("wv", [D, HD], bf16, isOutput=False)
    wo = nc.declare_dram_parameter("wo", [D, D], bf16, isOutput=False)
    bqk = nc.declare_dram_parameter("bqk", [128, 1], f32, isOutput=False)
    bv = nc.declare_dram_parameter("bv", [HD, 1], f32, isOutput=False)
    bo = nc.declare_dram_parameter("bo", [D, 1], f32, isOutput=False)
    out = nc.declare_dram_parameter("out", [D, TS], f32, isOutput=True)

    mask_np = np.triu(np.ones((C, C), np.float32))  # mask[s,t] = s<=t
    iden_np = np.eye(HD, dtype=ml_dtypes.bfloat16)
    iden128_np = np.eye(C, dtype=ml_dtypes.bfloat16)
    mask_d = nc.inline_tensor(mask_np, "causal_mask")
    iden_d = nc.inline_tensor(iden_np, "iden64")
    iden128_d = nc.inline_tensor(iden128_np, "iden128")

    with tile.TileContext(nc) as tc:
        with (
            tc.tile_pool(name="consts", bufs=1) as consts,
            tc.tile_pool(name="dram", bufs=1, space="DRAM") as dram,
        ):
            # ---- resident SBUF tensors (matmul operands in bf16) ----
            xt_sb = consts.tile([128, NF, BT], bf16)
            wa_sb = consts.tile([128, NF, 128], bf16)
            wv_sb = consts.tile([128, NF, HD], bf16)
            wo_sb = consts.tile([128, NF, D], bf16)
            bqk_sb = consts.tile([128, 1], f32)
            bv_sb = consts.tile([HD, 1], f32)
            bo_sb = consts.tile([128, NF], f32)
            mask_sb = consts.tile([C, C], f32)
            iden_sb = consts.tile([HD, HD], bf16)
            iden128_sb = consts.tile([C, C], bf16)
            qk_phi = consts.tile([128, BT], bf16)      # rows 0-63 qT, 64-127 kT
            k_sep = consts.tile([HD, BT], bf16)        # kT re-based to partition 0
            vT_sb = consts.tile([HD, BT], bf16)
            k_nrm = consts.tile([128, NCH, HD], bf16)
            v_aug = consts.tile([128, NCH, HD + 1], bf16)
            g_sb = [consts.tile([128, NF, C], bf16, tag=f"g{b}", name=f"g{b}")
                    for b in range(B)]
            sm_all = consts.tile([C, NCH, C], bf16)

            # per-batch bounce buffers: shard j = my head's out.T t-cols
            # [128j:128j+128) of that batch
            cc_in = [dram.tile([8, HD, C], bf16, tag=f"ci{b}", name=f"ci{b}")
                     for b in range(B)]
            cc_out = [dram.tile([8, HD, C], bf16, tag=f"co{b}", name=f"co{b}")
                      for b in range(B)]

            # xT on sync HWDGE, everything else on scalar HWDGE
            for tcol in range(NT):
                for f in range(NF):
                    nc.sync.dma_start(
                        xt_sb[:, f, ts(tcol, 512)],
                        xT[128 * f:128 * (f + 1), ts(tcol, 512)])
            for f in range(NF):
                nc.scalar.dma_start(wa_sb[:, f, :], wa[128 * f:128 * (f + 1), :])
                nc.scalar.dma_start(wv_sb[:, f, :], wv[128 * f:128 * (f + 1), :])
            for f in range(NF):
                nc.scalar.dma_start(wo_sb[:, f, :], wo[128 * f:128 * (f + 1), :])
            nc.scalar.dma_start(bqk_sb[:], bqk[:, :])
            nc.scalar.dma_start(bv_sb[:], bv[:, :])
            nc.scalar.dma_start(bo_sb[:], bo.ap().rearrange("(e p) o -> p (e o)", p=128))
            nc.scalar.dma_start(mask_sb[:], mask_d[:, :])
            nc.scalar.dma_start(iden_sb[:], iden_d[:, :])
            nc.scalar.dma_start(iden128_sb[:], iden128_d[:, :])
            nc.vector.memset(v_aug[:, :, HD:HD + 1], 1.0)

            # ---- phase 1: projections + phi + transposes + scores ----
            with (
                tc.tile_pool(name="psA", bufs=2, space="PSUM") as psA,
                tc.tile_pool(name="psB", bufs=2, space="PSUM") as psB,
                tc.tile_pool(name="psT", bufs=2, space="PSUM") as psT,
                tc.tile_pool(name="psSc", bufs=2, space="PSUM") as psSc,
                tc.tile_pool(name="ptmp", bufs=2) as ptmp,
            ):
                for tt in range(NT):
                    sl = ts(tt, 512)
                    pa = psA.tile([128, 512], f32)
                    pb = psB.tile([HD, 512], f32)
                    for f in range(NF):
                        nc.tensor.matmul(pa, wa_sb[:, f, :], xt_sb[:, f, sl],
                                         start=(f == 0), stop=(f == NF - 1))
                    for f in range(NF):
                        nc.tensor.matmul(pb, wv_sb[:, f, :], xt_sb[:, f, sl],
                                         start=(f == 0), stop=(f == NF - 1))
                    nc.scalar.activation(vT_sb[:, sl], pb, AF.Identity, bias=bv_sb[:])
                    # phi = exp(min(z,0)) + max(z,0), z = qk + bias (fused 2-op)
                    mm = ptmp.tile([128, 512], f32, tag="mm")
                    rr = ptmp.tile([128, 512], f32, tag="rr")
                    ee = ptmp.tile([128, 512], f32, tag="ee")
                    nc.vector.tensor_scalar(mm, pa, bqk_sb[:], 0.0,
                                            op0=ALU.add, op1=ALU.min)
                    nc.scalar.activation(ee, mm, AF.Exp)
                    nc.vector.tensor_scalar(rr, pa, bqk_sb[:], 0.0,
                                            op0=ALU.add, op1=ALU.max)
                    nc.vector.tensor_add(qk_phi[:, sl], ee, rr)
                    # re-base kT rows 64-127 -> partition 0 (SBUF->SBUF DMA)
                    nc.sync.dma_start(k_sep[:, sl], qk_phi[64:128, sl])

                    # transposes + masked scores for the 4 chunks in this t-tile
                    for jj in range(4):
                        i = tt * 4 + jj
                        cs = ts(i, C)
                        pt = psT.tile([C, HD], bf16, tag="tr")
                        nc.tensor.transpose(pt, k_sep[:, cs], iden_sb[:])
                        nc.scalar.copy(k_nrm[:, i, :], pt)
                        pv = psT.tile([C, HD], bf16, tag="tr")
                        nc.tensor.transpose(pv, vT_sb[:, cs], iden_sb[:])
                        nc.vector.tensor_copy(v_aug[:, i, 0:HD], pv)
                        ps = psSc.tile([C, C], f32, tag="ps")
                        nc.tensor.matmul(ps, k_sep[:, cs], qk_phi[0:64, cs],
                                         start=True, stop=True)
                        nc.vector.tensor_mul(sm_all[:, i, :], ps, mask_sb[:])

            # ---- phase 2+3: per-batch chain -> A2A -> o-proj ----
            with (
                tc.tile_pool(name="psS", bufs=1, space="PSUM") as psS,
                tc.tile_pool(name="psO", bufs=2, space="PSUM") as psO,
                tc.tile_pool(name="psTr", bufs=2, space="PSUM") as psTr,
                tc.tile_pool(name="psF", bufs=2, space="PSUM") as psF,
                tc.tile_pool(name="psW", bufs=1, space="PSUM") as psW,
                tc.tile_pool(name="attn", bufs=4) as attn,
                tc.tile_pool(name="fin", bufs=2) as fin,
            ):
                for b in range(B):
                    S = psS.tile([HD, HD + 1], f32, tag="S", name=f"S{b}")
                    for j in range(CPB):
                        i = b * CPB + j
                        cs = ts(i, C)
                        po = psO.tile([C, HD + 1], f32, tag="po")
                        if j == 0:
                            nc.tensor.matmul(po, sm_all[:, i, :], v_aug[:, i, :],
                                             start=True, stop=True)
                        else:
                            ssb = attn.tile([HD, HD + 1], bf16, tag="ssb")
                            nc.scalar.copy(ssb, S)
                            nc.tensor.matmul(po, sm_all[:, i, :], v_aug[:, i, :],
                                             start=True, stop=False)
                            nc.tensor.matmul(po, qk_phi[0:64, cs], ssb,
                                             start=False, stop=True)
                        if j < CPB - 1:
                            nc.tensor.matmul(S, k_nrm[:, i, :], v_aug[:, i, :],
                                             start=(j == 0), stop=(j == CPB - 2))
                        dn = attn.tile([C, 1], f32, tag="dn")
                        nc.vector.tensor_scalar_max(dn, po[:, HD:HD + 1], 1e-6)
                        dr = attn.tile([C, 1], f32, tag="dr")
                        nc.vector.reciprocal(dr, dn)
                        on = attn.tile([C, HD], bf16, tag="on")
                        nc.vector.tensor_scalar_mul(on, po[:, 0:HD], dr)
                        ptr = psTr.tile([HD, C], bf16, tag="ptr")
                        nc.tensor.transpose(ptr, on, iden128_sb[:])
                        ot = attn.tile([HD, C], bf16, tag="ot")
                        nc.scalar.copy(ot, ptr)
                        nc.sync.dma_start(cc_in[b][j, :, :], ot)
                    nc.gpsimd.collective_compute(
                        "AllToAll",
                        mybir.AluOpType.bypass,
                        replica_groups=[list(range(8))],
                        ins=[cc_in[b].opt()],
                        outs=[cc_out[b].opt()],
                    )

                # PE-warming dummies: chained so they trickle through idle
                # windows while the collectives run (results never read)
                pw = psW.tile([128, 512], f32)
                for w in range(N_WARM):
                    nc.tensor.matmul(pw, xt_sb[:, 0, 0:128], xt_sb[:, 1, 0:512],
                                     start=(w == 0), stop=(w == N_WARM - 1))

                for b in range(B):
                    for h in range(H):
                        nc.scalar.dma_start(
                            g_sb[b][64 * (h % 2):64 * (h % 2) + 64, h // 2, :],
                            cc_out[b][h, :, :])
                    for e in range(4):
                        pf = psF.tile([128, C], f32, tag="pf")
                        for ki in range(NF):
                            nc.tensor.matmul(pf, wo_sb[:, ki, 128 * e:128 * (e + 1)],
                                             g_sb[b][:, ki, :],
                                             start=(ki == 0), stop=(ki == NF - 1))
                        osl = fin.tile([128, C], f32, tag="osl")
                        nc.vector.tensor_scalar_add(osl, pf, bo_sb[:, e:e + 1])
                        nc.sync.dma_start(
                            out[128 * e:128 * (e + 1), C * b:C * (b + 1)], osl)

    nc.compile()
    _CACHE["nc"] = nc
    return nc


def _in_maps(x, Wq, bq, Wk, bk, Wv, bv, Wo, bo):
    import ml_dtypes
    bf = ml_dtypes.bfloat16
    x2 = np.ascontiguousarray(x.reshape(BT, D).T).astype(bf)
    woT = np.ascontiguousarray(Wo.T).astype(bf)
    bo_c = np.ascontiguousarray(bo.reshape(D, 1)).astype(np.float32)
    maps = []
    for c in range(8):
        sl = slice(HD * c, HD * (c + 1))
        maps.append(dict(
            xT=x2,
            wa=np.ascontiguousarray(np.concatenate([Wq[sl], Wk[sl]], 0).T).astype(bf),
            wv=np.ascontiguousarray(Wv[sl].T).astype(bf),
            wo=woT,
            bqk=np.ascontiguousarray(np.concatenate([bq[sl], bk[sl]]).reshape(128, 1)).astype(np.float32),
            bv=np.ascontiguousarray(bv[sl].reshape(HD, 1)).astype(np.float32),
            bo=bo_c,
        ))
    return maps


def kernel(x, Wq, bq, Wk, bk, Wv, bv, Wo, bo):
    from concourse import bass_utils

    nc = _build()
    maps = _in_maps(np.asarray(x), np.asarray(Wq), np.asarray(bq),
                    np.asarray(Wk), np.asarray(bk), np.asarray(Wv),
                    np.asarray(bv), np.asarray(Wo), np.asarray(bo))
    res = bass_utils.run_bass_kernel_spmd(nc, maps, core_ids=list(range(8)))
    o = np.zeros((BT, D), np.float32)
    for c in range(8):
        s = res.results[c]["out"]                     # (512, 256) o.T slices
        o[C * c:C * (c + 1), :] = s[:, 0:C].T         # batch 0 cols
        o[T + C * c:T + C * (c + 1), :] = s[:, C:2 * C].T  # batch 1 cols
    return np.ascontiguousarray(o.reshape(B, T, D)).astype(np.float32)


# revision 52
# speedup vs baseline: 1.1955x; 1.1955x over previous
"""Trainium2 Bass kernel for AssociativeMemoryModule (causal linear attention).

Sharding: head-parallel — core c owns head c for both batches. Each core:
  1. projects full x (pre-transposed, bf16 on host) to [q.T;k.T] and v.T,
  2. applies phi = elu+1 = min(exp(z),1) + relu(z) in f32,
  3. PE-transposes k_phi / v tiles to normal layout (bf16),
  4. runs chunked causal linear attention (C=128): per-chunk kv outer
     products + DVE prefix scan for the running state (no serial PE
     chain), normal-orientation epilogue (per-partition denominator),
     PE-transpose to out.T,
  5. per-batch AllToAll redistributes head-sharded -> t-sharded (bf16);
     batch-0's collective overlaps batch-1 compute, o-proj of batch 0
     overlaps batch-1's collective,
  6. each core computes the o-projection for its 128-col slice per batch.
Host reassembles the 8 (512, 256) o.T slices.
"""
import sys

import numpy as np

sys.path.insert(0, "/opt/trn_rl_repo")

H, HD, D = 8, 64, 512
B, T = 2, 1024
BT = B * T            # 2048
C = 128               # attention chunk
NCH = BT // C         # 16 chunks total
CPB = T // C          # 8 chunks per batch
TS = BT // 8          # 256: per-core output t-cols (128 per batch)
NF = D // 128         # 4 feature tiles
NT = 4                # t-tiles of 512 for projections

_CACHE = {}


def _build():
    if "nc" in _CACHE:
        return _CACHE["nc"]
    import concourse.mybir as mybir
    import concourse.tile as tile
    from concourse import bacc
    from concourse.bass import ts

    import ml_dtypes

    f32 = mybir.dt.float32
    bf16 = mybir.dt.bfloat16
    AF = mybir.ActivationFunctionType
    ALU = mybir.AluOpType

    nc = bacc.Bacc("TRN2", target_bir_lowering=False, debug=False, num_devices=8)

    xT = nc.declare_dram_parameter("xT", [D, BT], bf16, isOutput=False)
    wa = nc.declare_dram_parameter("wa", [D, 128], bf16, isOutput=False)
    wv = nc.declare_dram_parameter("wv", [D, HD], bf16, isOutput=False)
    wo = nc.declare_dram_parameter("wo", [D, D], bf16, isOutput=False)
    bqk = nc.declare_dram_parameter("bqk", [128, 1], f32, isOutput=False)
    bv = nc.declare_dram_parameter("bv", [HD, 1], f32, isOutput=False)
    bo = nc.declare_dram_parameter("bo", [D, 1], f32, isOutput=False)
    out = nc.declare_dram_parameter("out", [D, TS], f32, isOutput=True)

    mask_np = np.triu(np.ones((C, C), np.float32))  # mask[s,t] = s<=t
    iden_np = np.eye(HD, dtype=ml_dtypes.bfloat16)
    iden128_np = np.eye(C, dtype=ml_dtypes.bfloat16)
    mask_d = nc.inline_tensor(mask_np, "causal_mask")
    iden_d = nc.inline_tensor(iden_np, "iden64")
    iden128_d = nc.inline_tensor(iden128_np, "iden128")

    with tile.TileContext(nc) as tc:
        with (
            tc.tile_pool(name="consts", bufs=1) as consts,
            tc.tile_pool(name="dram", bufs=1, space="DRAM") as dram,
        ):
            # ---- resident SBUF tensors (matmul operands in bf16) ----
            xt_sb = consts.tile([128, NF, BT], bf16)
            wa_sb = consts.tile([128, NF, 128], bf16)
            wv_sb = consts.tile([128, NF, HD], bf16)
            wo_sb = consts.tile([128, NF, D], bf16)
            bqk_sb = consts.tile([128, 1], f32)
            bv_sb = consts.tile([HD, 1], f32)
            bo_sb = consts.tile([128, NF], f32)
            mask_sb = consts.tile([C, C], f32)
            iden_sb = consts.tile([HD, HD], bf16)
            iden128_sb = consts.tile([C, C], bf16)
            qk_phi = consts.tile([128, BT], bf16)      # rows 0-63 qT, 64-127 kT
            k_sep = consts.tile([HD, BT], bf16)        # kT re-based to partition 0
            vT_sb = consts.tile([HD, BT], bf16)
            k_nrm = consts.tile([128, NCH, HD], bf16)
            v_aug = consts.tile([128, NCH, HD + 1], bf16)
            g_sb = [consts.tile([128, NF, C], bf16, tag=f"g{b}", name=f"g{b}")
                    for b in range(B)]
            sm_all = consts.tile([C, NCH, C], bf16)
            kv_sb = consts.tile([HD, B, CPB - 1, HD + 1], f32)
            Sf = consts.tile([HD, B, CPB - 1, HD + 1], f32)
            Sb16 = consts.tile([HD, B, CPB - 1, HD + 1], bf16)

            # per-batch bounce buffers: shard j = my head's out.T t-cols
            # [128j:128j+128) of that batch
            cc_in = [dram.tile([8, HD, C], bf16, tag=f"ci{b}", name=f"ci{b}")
                     for b in range(B)]
            cc_out = [dram.tile([8, HD, C], bf16, tag=f"co{b}", name=f"co{b}")
                      for b in range(B)]

            # xT issue split between sync HWDGE and gpsimd SWDGE queues
            for tcol in range(NT):
                for f in range(NF):
                    eng = nc.sync if (f % 2 == 0) else nc.gpsimd
                    eng.dma_start(
                        xt_sb[:, f, ts(tcol, 512)],
                        xT[128 * f:128 * (f + 1), ts(tcol, 512)])
            for f in range(NF):
                nc.scalar.dma_start(wa_sb[:, f, :], wa[128 * f:128 * (f + 1), :])
                nc.scalar.dma_start(wv_sb[:, f, :], wv[128 * f:128 * (f + 1), :])
            for f in range(NF):
                nc.scalar.dma_start(wo_sb[:, f, :], wo[128 * f:128 * (f + 1), :])
            nc.scalar.dma_start(bqk_sb[:], bqk[:, :])
            nc.scalar.dma_start(bv_sb[:], bv[:, :])
            nc.scalar.dma_start(bo_sb[:], bo.ap().rearrange("(e p) o -> p (e o)", p=128))
            nc.scalar.dma_start(mask_sb[:], mask_d[:, :])
            nc.scalar.dma_start(iden_sb[:], iden_d[:, :])
            nc.scalar.dma_start(iden128_sb[:], iden128_d[:, :])
            nc.vector.memset(v_aug[:, :, HD:HD + 1], 1.0)

            # ---- interleaved phases: [tt0,tt1] -> b0 attn+A2A#1 ->
            # [tt2,tt3] -> b1 attn+A2A#2 -> o-proj b0 -> o-proj b1.
            # Collective entry latency and rank skew overlap compute.
            with (
                tc.tile_pool(name="psA", bufs=2, space="PSUM") as psA,
                tc.tile_pool(name="psB", bufs=1, space="PSUM") as psB,
                tc.tile_pool(name="psT", bufs=2, space="PSUM") as psT,
                tc.tile_pool(name="psSc", bufs=1, space="PSUM") as psSc,
                tc.tile_pool(name="psO", bufs=2, space="PSUM") as psO,
                tc.tile_pool(name="ptmp", bufs=3) as ptmp,
                tc.tile_pool(name="attn", bufs=6) as attn,
                tc.tile_pool(name="fin", bufs=3) as fin,
            ):
                def proj_tile(tt):
                    sl = ts(tt, 512)
                    pa = psA.tile([128, 512], f32, tag="pa", name=f"pa{tt}")
                    pb = psB.tile([HD, 512], f32, tag="pb", name=f"pb{tt}")
                    for f in range(NF):
                        nc.tensor.matmul(pa, wa_sb[:, f, :], xt_sb[:, f, sl],
                                         start=(f == 0), stop=(f == NF - 1))
                    for f in range(NF):
                        nc.tensor.matmul(pb, wv_sb[:, f, :], xt_sb[:, f, sl],
                                         start=(f == 0), stop=(f == NF - 1))
                    nc.scalar.activation(vT_sb[:, sl], pb, AF.Identity, bias=bv_sb[:])
                    # phi = exp(min(z,0)) + relu(z) = min(exp(z),1) + relu(z):
                    # both ACT ops read PSUM directly with fused bias
                    rr = ptmp.tile([128, 512], f32, tag="rr", name=f"rr{tt}")
                    ee = ptmp.tile([128, 512], f32, tag="ee", name=f"ee{tt}")
                    mm = ptmp.tile([128, 512], f32, tag="mm", name=f"mm{tt}")
                    nc.scalar.activation(ee, pa, AF.Exp, bias=bqk_sb[:])
                    nc.scalar.activation(rr, pa, AF.Relu, bias=bqk_sb[:])
                    nc.vector.tensor_scalar_min(mm, ee, 1.0)
                    nc.vector.tensor_add(qk_phi[:, sl], mm, rr)
                    # re-base kT rows 64-127 -> partition 0 (SBUF->SBUF DMA)
                    nc.sync.dma_start(k_sep[:, sl], qk_phi[64:128, sl])
                    # transposes + masked scores for the 4 chunks in this t-tile
                    for jj in range(4):
                        i = tt * 4 + jj
                        cs = ts(i, C)
                        pt = psT.tile([C, HD], bf16, tag="tr", name=f"pt{i}")
                        nc.tensor.transpose(pt, k_sep[:, cs], iden_sb[:])
                        nc.scalar.copy(k_nrm[:, i, :], pt)
                        pv = psT.tile([C, HD], bf16, tag="tr", name=f"pv{i}")
                        nc.tensor.transpose(pv, vT_sb[:, cs], iden_sb[:])
                        nc.scalar.copy(v_aug[:, i, 0:HD], pv)
                        ps = psSc.tile([C, C], f32, tag="ps", name=f"ps{i}")
                        nc.tensor.matmul(ps, k_sep[:, cs], qk_phi[0:64, cs],
                                         start=True, stop=True)
                        nc.vector.tensor_mul(sm_all[:, i, :], ps, mask_sb[:])

                def kv_part(b, jlo, jhi):
                    # kv products + incremental prefix for j in [jlo, jhi)
                    for j in range(jlo, min(jhi, CPB - 1)):
                        i = b * CPB + j
                        pkv = psO.tile([HD, HD + 1], f32, tag="po", name=f"pkv{i}")
                        nc.tensor.matmul(pkv, k_nrm[:, i, :], v_aug[:, i, :],
                                         start=True, stop=True)
                        nc.scalar.copy(kv_sb[:, b, j, :], pkv)
                        if j == 0:
                            nc.vector.tensor_copy(Sf[:, b, 0, :], kv_sb[:, b, 0, :])
                            nc.scalar.copy(Sb16[:, b, 0, :], kv_sb[:, b, 0, :])
                        else:
                            nc.vector.tensor_add(Sf[:, b, j, :], Sf[:, b, j - 1, :],
                                                 kv_sb[:, b, j, :])
                            nc.scalar.copy(Sb16[:, b, j, :], Sf[:, b, j, :])

                def po_part(b, jlo, jhi):
                    for j in range(jlo, jhi):
                        i = b * CPB + j
                        cs = ts(i, C)
                        po = psO.tile([C, HD + 1], f32, tag="po", name=f"po{i}")
                        if j == 0:
                            nc.tensor.matmul(po, sm_all[:, i, :], v_aug[:, i, :],
                                             start=True, stop=True)
                        else:
                            nc.tensor.matmul(po, sm_all[:, i, :], v_aug[:, i, :],
                                             start=True, stop=False)
                            nc.tensor.matmul(po, qk_phi[0:64, cs],
                                             Sb16[:, b, j - 1, :],
                                             start=False, stop=True)
                        # denom > 0 always (phi > 0); reference's 1e-6 clamp
                        # can never bind at these magnitudes
                        dr = attn.tile([C, 1], f32, tag="dr", name=f"dr{i}")
                        nc.vector.reciprocal(dr, po[:, HD:HD + 1])
                        on = attn.tile([C, HD], bf16, tag="on", name=f"on{i}")
                        nc.vector.tensor_scalar_mul(on, po[:, 0:HD], dr)
                        ptr = psT.tile([HD, C], bf16, tag="tr", name=f"ptr{i}")
                        nc.tensor.transpose(ptr, on, iden128_sb[:])
                        ot = attn.tile([HD, C], bf16, tag="ot", name=f"ot{i}")
                        nc.vector.tensor_copy(ot, ptr)
                        nc.sync.dma_start(cc_in[b][j, :, :], ot)

                def trigger(b):
                    nc.gpsimd.collective_compute(
                        "AllToAll",
                        mybir.AluOpType.bypass,
                        replica_groups=[list(range(8))],
                        ins=[cc_in[b].opt()],
                        outs=[cc_out[b].opt()],
                    )

                def oproj_batch(b):
                    # gathered heads, partition-packed in pairs: 2 DMAs/batch
                    co = cc_out[b]
                    nc.scalar.dma_start(
                        g_sb[b][0:64, :, :],
                        co.rearrange("(ki two) m t -> two m ki t", two=2)[0])
                    nc.scalar.dma_start(
                        g_sb[b][64:128, :, :],
                        co.rearrange("(ki two) m t -> two m ki t", two=2)[1])
                    for e in range(4):
                        pf = psSc.tile([128, C], f32, tag="ps", name=f"pf{b}{e}")
                        for ki in range(NF):
                            nc.tensor.matmul(pf, wo_sb[:, ki, 128 * e:128 * (e + 1)],
                                             g_sb[b][:, ki, :],
                                             start=(ki == 0), stop=(ki == NF - 1))
                        osl = fin.tile([128, C], f32, tag="osl", name=f"osl{b}{e}")
                        nc.scalar.activation(osl, pf, AF.Identity,
                                             bias=bo_sb[:, e:e + 1])
                        nc.sync.dma_start(
                            out[128 * e:128 * (e + 1), C * b:C * (b + 1)], osl)

                proj_tile(0)
                proj_tile(1)
                kv_part(0, 0, 7)
                po_part(0, 0, 8)
                trigger(0)
                proj_tile(2)
                proj_tile(3)
                kv_part(1, 0, 7)
                po_part(1, 0, 8)
                trigger(1)
                oproj_batch(0)
                oproj_batch(1)

    nc.compile()
    _CACHE["nc"] = nc
    return nc


def _in_maps(x, Wq, bq, Wk, bk, Wv, bv, Wo, bo):
    import ml_dtypes
    bf = ml_dtypes.bfloat16
    x2 = np.ascontiguousarray(x.reshape(BT, D).T).astype(bf)
    woT = np.ascontiguousarray(Wo.T).astype(bf)
    bo_c = np.ascontiguousarray(bo.reshape(D, 1)).astype(np.float32)
    maps = []
    for c in range(8):
        sl = slice(HD * c, HD * (c + 1))
        maps.append(dict(
            xT=x2,
            wa=np.ascontiguousarray(np.concatenate([Wq[sl], Wk[sl]], 0).T).astype(bf),
            wv=np.ascontiguousarray(Wv[sl].T).astype(bf),
            wo=woT,
            bqk=np.ascontiguousarray(np.concatenate([bq[sl], bk[sl]]).reshape(128, 1)).astype(np.float32),
            bv=np.ascontiguousarray(bv[sl].reshape(HD, 1)).astype(np.float32),
            bo=bo_c,
        ))
    return maps


def kernel(x, Wq, bq, Wk, bk, Wv, bv, Wo, bo):
    from concourse import bass_utils

    nc = _build()
    maps = _in_maps(np.asarray(x), np.asarray(Wq), np.asarray(bq),
                    np.asarray(Wk), np.asarray(bk), np.asarray(Wv),
                    np.asarray(bv), np.asarray(Wo), np.asarray(bo))
    res = bass_utils.run_bass_kernel_spmd(nc, maps, core_ids=list(range(8)))
    o = np.zeros((BT, D), np.float32)
    for c in range(8):
        s = res.results[c]["out"]                     # (512, 256) o.T slices
        o[C * c:C * (c + 1), :] = s[:, 0:C].T         # batch 0 cols
        o[T + C * c:T + C * (c + 1), :] = s[:, C:2 * C].T  # batch 1 cols
    return np.ascontiguousarray(o.reshape(B, T, D)).astype(np.float32)


# revision 54
# speedup vs baseline: 1.3280x; 1.1108x over previous
"""Trainium2 Bass kernel for AssociativeMemoryModule (causal linear attention).

Sharding: head-parallel — core c owns head c for both batches. Each core:
  1. projects full x (pre-transposed, bf16 on host) to [q.T;k.T] and v.T,
  2. applies phi = elu+1 = min(exp(z),1) + relu(z) in f32,
  3. PE-transposes k_phi / v tiles to normal layout (bf16),
  4. runs chunked causal linear attention (C=128): per-chunk kv outer
     products + DVE prefix scan for the running state (no serial PE
     chain), normal-orientation epilogue (per-partition denominator),
     PE-transpose to out.T,
  5. per-batch AllToAll redistributes head-sharded -> t-sharded (bf16);
     batch-0's collective overlaps batch-1 compute, o-proj of batch 0
     overlaps batch-1's collective,
  6. each core computes the o-projection for its 128-col slice per batch.
Host reassembles the 8 (512, 256) o.T slices.
"""
import sys

import numpy as np

sys.path.insert(0, "/opt/trn_rl_repo")

H, HD, D = 8, 64, 512
B, T = 2, 1024
BT = B * T            # 2048
C = 128               # attention chunk
NCH = BT // C         # 16 chunks total
CPB = T // C          # 8 chunks per batch
TS = BT // 8          # 256: per-core output t-cols (128 per batch)
NF = D // 128         # 4 feature tiles
NT = 4                # t-tiles of 512 for projections

_CACHE = {}


def _build():
    if "nc" in _CACHE:
        return _CACHE["nc"]
    import concourse.mybir as mybir
    import concourse.tile as tile
    from concourse import bacc
    from concourse.bass import ts

    import ml_dtypes

    f32 = mybir.dt.float32
    bf16 = mybir.dt.bfloat16
    AF = mybir.ActivationFunctionType
    ALU = mybir.AluOpType

    nc = bacc.Bacc("TRN2", target_bir_lowering=False, debug=False, num_devices=8,
                   num_swdge_queues=4)

    xT = nc.declare_dram_parameter("xT", [D, BT], bf16, isOutput=False)
    wa = nc.declare_dram_parameter("wa", [D, 128], bf16, isOutput=False)
    wv = nc.declare_dram_parameter("wv", [D, HD], bf16, isOutput=False)
    wo = nc.declare_dram_parameter("wo", [D, D], bf16, isOutput=False)
    bqk = nc.declare_dram_parameter("bqk", [128, 1], f32, isOutput=False)
    bv = nc.declare_dram_parameter("bv", [HD, 1], f32, isOutput=False)
    bo = nc.declare_dram_parameter("bo", [D, 1], f32, isOutput=False)
    out = nc.declare_dram_parameter("out", [D, TS], f32, isOutput=True)

    mask_np = np.triu(np.ones((C, C), np.float32))  # mask[s,t] = s<=t
    iden_np = np.eye(HD, dtype=ml_dtypes.bfloat16)
    iden128_np = np.eye(C, dtype=ml_dtypes.bfloat16)
    mask_d = nc.inline_tensor(mask_np, "causal_mask")
    iden_d = nc.inline_tensor(iden_np, "iden64")
    iden128_d = nc.inline_tensor(iden128_np, "iden128")

    with tile.TileContext(nc) as tc:
        with (
            tc.tile_pool(name="consts", bufs=1) as consts,
            tc.tile_pool(name="dram", bufs=1, space="DRAM") as dram,
        ):
            # ---- resident SBUF tensors (matmul operands in bf16) ----
            xt_sb = consts.tile([128, NF, BT], bf16)
            wa_sb = consts.tile([128, NF, 128], bf16)
            wv_sb = consts.tile([128, NF, HD], bf16)
            wo_sb = consts.tile([128, NF, D], bf16)
            bqk_sb = consts.tile([128, 1], f32)
            bv_sb = consts.tile([HD, 1], f32)
            bo_sb = consts.tile([128, NF], f32)
            mask_sb = consts.tile([C, C], f32)
            iden_sb = consts.tile([HD, HD], bf16)
            iden128_sb = consts.tile([C, C], bf16)
            qk_phi = consts.tile([128, BT], bf16)      # rows 0-63 qT, 64-127 kT
            k_sep = consts.tile([HD, BT], bf16)        # kT re-based to partition 0
            vT_sb = consts.tile([HD, BT], bf16)
            k_nrm = consts.tile([128, NCH, HD], bf16)
            v_aug = consts.tile([128, NCH, HD + 1], bf16)
            g_sb = [consts.tile([128, NF, C], bf16, tag=f"g{b}", name=f"g{b}")
                    for b in range(B)]
            sm_all = consts.tile([C, NCH, C], bf16)
            kv_sb = consts.tile([HD, B, CPB - 1, HD + 1], f32)
            Sf = consts.tile([HD, B, CPB - 1, HD + 1], f32)
            Sb16 = consts.tile([HD, B, CPB - 1, HD + 1], bf16)

            # per-batch bounce buffers: shard j = my head's out.T t-cols
            # [128j:128j+128) of that batch
            cc_in = [dram.tile([8, HD, C], bf16, tag=f"ci{b}", name=f"ci{b}")
                     for b in range(B)]
            cc_out = [dram.tile([8, HD, C], bf16, tag=f"co{b}", name=f"co{b}")
                      for b in range(B)]

            # xT issue split between sync HWDGE and gpsimd SWDGE queues
            for tcol in range(NT):
                for f in range(NF):
                    eng = nc.sync if (f % 2 == 0) else nc.gpsimd
                    eng.dma_start(
                        xt_sb[:, f, ts(tcol, 512)],
                        xT[128 * f:128 * (f + 1), ts(tcol, 512)])
            for f in range(NF):
                nc.scalar.dma_start(wa_sb[:, f, :], wa[128 * f:128 * (f + 1), :])
                nc.scalar.dma_start(wv_sb[:, f, :], wv[128 * f:128 * (f + 1), :])
            for f in range(NF):
                nc.scalar.dma_start(wo_sb[:, f, :], wo[128 * f:128 * (f + 1), :])
            nc.scalar.dma_start(bqk_sb[:], bqk[:, :])
            nc.scalar.dma_start(bv_sb[:], bv[:, :])
            nc.scalar.dma_start(bo_sb[:], bo.ap().rearrange("(e p) o -> p (e o)", p=128))
            nc.scalar.dma_start(mask_sb[:], mask_d[:, :])
            nc.scalar.dma_start(iden_sb[:], iden_d[:, :])
            nc.scalar.dma_start(iden128_sb[:], iden128_d[:, :])
            nc.vector.memset(v_aug[:, :, HD:HD + 1], 1.0)

            # ---- interleaved phases: [tt0,tt1] -> b0 attn+A2A#1 ->
            # [tt2,tt3] -> b1 attn+A2A#2 -> o-proj b0 -> o-proj b1.
            # Collective entry latency and rank skew overlap compute.
            with (
                tc.tile_pool(name="psA", bufs=2, space="PSUM") as psA,
                tc.tile_pool(name="psB", bufs=1, space="PSUM") as psB,
                tc.tile_pool(name="psT", bufs=2, space="PSUM") as psT,
                tc.tile_pool(name="psSc", bufs=1, space="PSUM") as psSc,
                tc.tile_pool(name="psO", bufs=2, space="PSUM") as psO,
                tc.tile_pool(name="ptmp", bufs=3) as ptmp,
                tc.tile_pool(name="attn", bufs=6) as attn,
                tc.tile_pool(name="fin", bufs=3) as fin,
            ):
                def proj_tile(tt):
                    sl = ts(tt, 512)
                    pa = psA.tile([128, 512], f32, tag="pa", name=f"pa{tt}")
                    pb = psB.tile([HD, 512], f32, tag="pb", name=f"pb{tt}")
                    for f in range(NF):
                        nc.tensor.matmul(pa, wa_sb[:, f, :], xt_sb[:, f, sl],
                                         start=(f == 0), stop=(f == NF - 1))
                    for f in range(NF):
                        nc.tensor.matmul(pb, wv_sb[:, f, :], xt_sb[:, f, sl],
                                         start=(f == 0), stop=(f == NF - 1))
                    nc.scalar.activation(vT_sb[:, sl], pb, AF.Identity, bias=bv_sb[:])
                    # phi = exp(min(z,0)) + relu(z) = min(exp(z),1) + relu(z):
                    # both ACT ops read PSUM directly with fused bias
                    rr = ptmp.tile([128, 512], f32, tag="rr", name=f"rr{tt}")
                    ee = ptmp.tile([128, 512], f32, tag="ee", name=f"ee{tt}")
                    mm = ptmp.tile([128, 512], f32, tag="mm", name=f"mm{tt}")
                    nc.scalar.activation(ee, pa, AF.Exp, bias=bqk_sb[:])
                    nc.scalar.activation(rr, pa, AF.Relu, bias=bqk_sb[:])
                    nc.vector.tensor_scalar_min(mm, ee, 1.0)
                    nc.vector.tensor_add(qk_phi[:, sl], mm, rr)
                    # re-base kT rows 64-127 -> partition 0 (SBUF->SBUF DMA)
                    nc.sync.dma_start(k_sep[:, sl], qk_phi[64:128, sl])
                    # transposes + masked scores for the 4 chunks in this t-tile
                    for jj in range(4):
                        i = tt * 4 + jj
                        cs = ts(i, C)
                        pt = psT.tile([C, HD], bf16, tag="tr", name=f"pt{i}")
                        nc.tensor.transpose(pt, k_sep[:, cs], iden_sb[:])
                        nc.scalar.copy(k_nrm[:, i, :], pt)
                        pv = psT.tile([C, HD], bf16, tag="tr", name=f"pv{i}")
                        nc.tensor.transpose(pv, vT_sb[:, cs], iden_sb[:])
                        nc.scalar.copy(v_aug[:, i, 0:HD], pv)
                        ps = psSc.tile([C, C], f32, tag="ps", name=f"ps{i}")
                        nc.tensor.matmul(ps, k_sep[:, cs], qk_phi[0:64, cs],
                                         start=True, stop=True)
                        nc.vector.tensor_mul(sm_all[:, i, :], ps, mask_sb[:])

                def kv_part(b, jlo, jhi):
                    # kv products + incremental prefix for j in [jlo, jhi)
                    for j in range(jlo, min(jhi, CPB - 1)):
                        i = b * CPB + j
                        pkv = psO.tile([HD, HD + 1], f32, tag="po", name=f"pkv{i}")
                        nc.tensor.matmul(pkv, k_nrm[:, i, :], v_aug[:, i, :],
                                         start=True, stop=True)
                        nc.scalar.copy(kv_sb[:, b, j, :], pkv)
                        if j == 0:
                            nc.vector.tensor_copy(Sf[:, b, 0, :], kv_sb[:, b, 0, :])
                            nc.scalar.copy(Sb16[:, b, 0, :], kv_sb[:, b, 0, :])
                        else:
                            nc.vector.tensor_add(Sf[:, b, j, :], Sf[:, b, j - 1, :],
                                                 kv_sb[:, b, j, :])
                            nc.scalar.copy(Sb16[:, b, j, :], Sf[:, b, j, :])

                def po_part(b, jlo, jhi):
                    for j in range(jlo, jhi):
                        i = b * CPB + j
                        cs = ts(i, C)
                        po = psO.tile([C, HD + 1], f32, tag="po", name=f"po{i}")
                        if j == 0:
                            nc.tensor.matmul(po, sm_all[:, i, :], v_aug[:, i, :],
                                             start=True, stop=True)
                        else:
                            nc.tensor.matmul(po, sm_all[:, i, :], v_aug[:, i, :],
                                             start=True, stop=False)
                            nc.tensor.matmul(po, qk_phi[0:64, cs],
                                             Sb16[:, b, j - 1, :],
                                             start=False, stop=True)
                        # denom > 0 always (phi > 0); reference's 1e-6 clamp
                        # can never bind at these magnitudes
                        dr = attn.tile([C, 1], f32, tag="dr", name=f"dr{i}")
                        nc.vector.reciprocal(dr, po[:, HD:HD + 1])
                        on = attn.tile([C, HD], bf16, tag="on", name=f"on{i}")
                        nc.vector.tensor_scalar_mul(on, po[:, 0:HD], dr)
                        ptr = psT.tile([HD, C], bf16, tag="tr", name=f"ptr{i}")
                        nc.tensor.transpose(ptr, on, iden128_sb[:])
                        ot = attn.tile([HD, C], bf16, tag="ot", name=f"ot{i}")
                        nc.vector.tensor_copy(ot, ptr)
                        nc.sync.dma_start(cc_in[b][j, :, :], ot)

                def trigger(b):
                    nc.gpsimd.collective_compute(
                        "AllToAll",
                        mybir.AluOpType.bypass,
                        replica_groups=[list(range(8))],
                        ins=[cc_in[b].opt()],
                        outs=[cc_out[b].opt()],
                    )

                def oproj_batch(b):
                    # gathered heads, partition-packed in pairs: 2 DMAs/batch
                    co = cc_out[b]
                    nc.scalar.dma_start(
                        g_sb[b][0:64, :, :],
                        co.rearrange("(ki two) m t -> two m ki t", two=2)[0])
                    nc.scalar.dma_start(
                        g_sb[b][64:128, :, :],
                        co.rearrange("(ki two) m t -> two m ki t", two=2)[1])
                    for e in range(4):
                        pf = psSc.tile([128, C], f32, tag="ps", name=f"pf{b}{e}")
                        for ki in range(NF):
                            nc.tensor.matmul(pf, wo_sb[:, ki, 128 * e:128 * (e + 1)],
                                             g_sb[b][:, ki, :],
                                             start=(ki == 0), stop=(ki == NF - 1))
                        osl = fin.tile([128, C], f32, tag="osl", name=f"osl{b}{e}")
                        nc.scalar.activation(osl, pf, AF.Identity,
                                             bias=bo_sb[:, e:e + 1])
                        nc.sync.dma_start(
                            out[128 * e:128 * (e + 1), C * b:C * (b + 1)], osl)

                proj_tile(0)
                proj_tile(1)
                kv_part(0, 0, 7)
                po_part(0, 0, 8)
                trigger(0)
                proj_tile(2)
                proj_tile(3)
                kv_part(1, 0, 7)
                po_part(1, 0, 8)
                trigger(1)
                oproj_batch(0)
                oproj_batch(1)

    nc.compile()
    _CACHE["nc"] = nc
    return nc


def _in_maps(x, Wq, bq, Wk, bk, Wv, bv, Wo, bo):
    import ml_dtypes
    bf = ml_dtypes.bfloat16
    x2 = np.ascontiguousarray(x.reshape(BT, D).T).astype(bf)
    woT = np.ascontiguousarray(Wo.T).astype(bf)
    bo_c = np.ascontiguousarray(bo.reshape(D, 1)).astype(np.float32)
    maps = []
    for c in range(8):
        sl = slice(HD * c, HD * (c + 1))
        maps.append(dict(
            xT=x2,
            wa=np.ascontiguousarray(np.concatenate([Wq[sl], Wk[sl]], 0).T).astype(bf),
            wv=np.ascontiguousarray(Wv[sl].T).astype(bf),
            wo=woT,
            bqk=np.ascontiguousarray(np.concatenate([bq[sl], bk[sl]]).reshape(128, 1)).astype(np.float32),
            bv=np.ascontiguousarray(bv[sl].reshape(HD, 1)).astype(np.float32),
            bo=bo_c,
        ))
    return maps


def kernel(x, Wq, bq, Wk, bk, Wv, bv, Wo, bo):
    from concourse import bass_utils

    nc = _build()
    maps = _in_maps(np.asarray(x), np.asarray(Wq), np.asarray(bq),
                    np.asarray(Wk), np.asarray(bk), np.asarray(Wv),
                    np.asarray(bv), np.asarray(Wo), np.asarray(bo))
    res = bass_utils.run_bass_kernel_spmd(nc, maps, core_ids=list(range(8)))
    o = np.zeros((BT, D), np.float32)
    for c in range(8):
        s = res.results[c]["out"]                     # (512, 256) o.T slices
        o[C * c:C * (c + 1), :] = s[:, 0:C].T         # batch 0 cols
        o[T + C * c:T + C * (c + 1), :] = s[:, C:2 * C].T  # batch 1 cols
    return np.ascontiguousarray(o.reshape(B, T, D)).astype(np.float32)
